# revision 17
# baseline (speedup 1.0000x reference)
"""Trainium2 Bass kernel for nn_EdgeEncoder (moe_routing).

Strategy
--------
Each of E edges is routed to 1 of 9 expert MLPs (4 -> 256 -> 256), then
  out = relu(concat([type_embed[tid], source_embed[sid], pv]) @ Wf + bf).

Host (numpy, cheap O(E) work):
  * scale/mask params, group edge indices by expert (base type) at
    256-edge granularity, split evenly over 8 cores (identical layout on
    every core so one SPMD program serves all 8),
  * algebraic fusions: b1 rides a ones-row inside layer 1;
    V[t] = W2[t] @ Wf_pv fuses layer 2 with the final projection;
    G[t] = [const; type_embed@Wf_t; source_embed@Wf_s] turns the
    embedding gathers + all biases into one small matmul against the
    one-hot rows.

Device, all bf16 operands (fp32 PSUM accumulate; rel-err gate is 2e-2):
  The per-512-edge block needs h = relu(W1e.T @ x1) (2 matmuls, K=5),
  the G part (2 matmuls, K=20) and V part (4 matmuls, K=128) of the
  output. The four small-K matmuls are packed into ONE matmul slot via
  tile_position row-strips (0/32/64/96): the host ships a [32, L] input
  with per-strip band contents (x+ones for the two L1 strips,
  ones+one-hots for the two K=20 G strips), and the strip matmuls run
  concurrently in the PE array. Per block: 1 packed slot + 4 V matmuls
  + a tiny HAM-keeper matmul that holds the PE clock gate at 8/8.
  Weights ship compactly (W4a rows 0-36, W4b rows 64-119) and deep SBUF
  pools (inp 6 / hsb 5 / osb 8) decouple the pipeline from DMA latency.
  relu-h is one [128,1024] ACT op, out-relu one [128,1024] DVE op
  (PSUM->SBUF, the only engines that can read PSUM). Outputs are stored
  bf16 in a DMA-native packed layout and unscrambled on host.
"""

import math
import os

import ml_dtypes
import numpy as np

import concourse.bacc as bacc
import concourse.bass as bass
import concourse.mybir as mybir
import concourse.tile as tile
from concourse.bass_utils import run_bass_kernel_spmd

# ---- static module configuration (mirrors the torch source) ----
T = 9            # base types ("experts")
P_MAX = 4
D = 256
N_TYPES = 14
N_SRC = 5
NCORES = 8
BLOCK = 512      # edges per block (one PSUM bank per 128-out-dims half)
GRAN = 1         # run granularity (expert segments padded to per-core exact)

BASE_MAP = np.array([0, 0, 0, 1, 1, 1, 2, 2, 3, 4, 5, 6, 7, 8], dtype=np.int32)
PCOUNT = np.array([2, 2, 1, 1, 1, 1, 3, 2, 4], dtype=np.int32)
SCALES = np.ones((T, P_MAX), dtype=np.float32)
SCALES[0, :2] = [1.0, 1e-06]      # nmos  m, w
SCALES[1, :2] = [1.0, 1e-06]      # pmos  m, w
SCALES[2, 0] = 1.0                # balun rout
SCALES[3, 0] = 1000.0             # resistor r
SCALES[4, 0] = 1e-12              # capacitor c
SCALES[5, 0] = 1e-09              # inductor l
SCALES[6, :3] = [1.0, 1.0, 1.0]   # vsource dc, mag, phase
SCALES[7, :2] = [0.001, 0.001]    # isource dc, mag
SCALES[8, :4] = [1.0, 1.0, 1e9, 1.0]  # port dbm, dc, freq, num

# xu strip layout (replicated at partition offsets 0/32/64/96):
#   rows 0-3: scaled params, row 4: ones (valid), rows 5-18: type one-hot,
#   rows 19-23: source one-hot, rows 24-31: zero
K_L1 = 5                   # x rows + ones
K_G = 20                   # G bands: ones + type one-hot + src one-hot
STRIPS = (0, 32, 64, 96)   # (L1 h0, L1 h1, G g0, G g1)

_F32 = mybir.dt.float32
_BF16 = mybir.dt.bfloat16
_WARM_BURST = int(os.environ.get("EDGEENC_WARM_BURST", "6"))
_FILL = int(os.environ.get("EDGEENC_FILL", "1"))
_FILLW = int(os.environ.get("EDGEENC_FILLW", "96"))

_PROGRAM_CACHE: dict = {}
LAST_RESULT = None  # BassKernelResults of the most recent run (for test harness)


def _layout(base_ids: np.ndarray):
    """Per-expert per-core segment sizes (multiples of GRAN), identical on
    every core so one program serves all 8."""
    n_t = np.bincount(base_ids, minlength=T)
    m_t = np.zeros(T, dtype=np.int64)
    for t in range(T):
        if n_t[t] > 0:
            per_core = math.ceil(n_t[t] / NCORES)
            m_t[t] = math.ceil(per_core / GRAN) * GRAN
    L0 = int(m_t.sum())
    L = math.ceil(L0 / BLOCK) * BLOCK
    # fold the tail pad into the last present expert's segment
    last = int(np.nonzero(m_t)[0][-1])
    m_t[last] += L - L0
    return n_t, m_t, L


def _build_order(base_ids: np.ndarray, n_t, m_t, L) -> np.ndarray:
    """ORD[c, j] = global edge index at per-core slot j (or -1 = pad)."""
    ORD = np.full((NCORES, L), -1, dtype=np.int64)
    off = 0
    for t in range(T):
        if m_t[t] == 0:
            continue
        seg = int(m_t[t])
        idx = np.nonzero(base_ids == t)[0]
        arr = np.full(NCORES * seg, -1, dtype=np.int64)
        arr[: idx.shape[0]] = idx
        ORD[:, off : off + seg] = arr.reshape(NCORES, seg)
        off += seg
    return ORD


def _host_inputs(type_ids, source_ids, params, ORD):
    """XU[c] = [32, L] bf16: x rows, ones, type one-hot, src one-hot, zeros."""
    base_ids = BASE_MAP[type_ids]
    scales = SCALES[base_ids]                                  # [E,4]
    validp = np.arange(P_MAX)[None, :] < PCOUNT[base_ids][:, None]
    x = np.where(validp, params.astype(np.float32) / scales, 0.0).astype(np.float32)

    L = ORD.shape[1]
    XU = np.zeros((NCORES, 128, L), dtype=np.float32)
    valid = ORD >= 0
    ids = ORD[valid]
    tmp = np.zeros((NCORES, L, P_MAX), dtype=np.float32)
    tmp[valid] = x[ids]
    # bands 0/1 (L1 strips): x rows + ones
    XU[:, 0:P_MAX, :] = tmp.transpose(0, 2, 1)
    XU[:, P_MAX, :] = valid
    XU[:, 32:37, :] = XU[:, 0:5, :]
    # bands 2/3 (G strips): ones + type one-hot + src one-hot
    XU[:, 64, :] = valid
    ci, co = np.nonzero(valid)
    XU[ci, 65 + type_ids[ids], co] = 1.0
    XU[ci, 79 + source_ids[ids], co] = 1.0
    XU[:, 96:116, :] = XU[:, 64:84, :]
    return XU.astype(ml_dtypes.bfloat16)


def _host_weights(type_embed, source_embed, W1, b1, W2, b2, Wf, bf):
    f = np.float32
    W1 = W1.astype(f); b1 = b1.astype(f); W2 = W2.astype(np.float64)
    b2 = b2.astype(f); Wf = Wf.astype(f); bf = bf.astype(f)
    type_embed = type_embed.astype(f); source_embed = source_embed.astype(f)

    Wft, Wfs, Wfp = Wf[:D], Wf[D : 2 * D], Wf[2 * D :]
    # V[t] = W2[t] @ Wf_pv (f64), fusing layer 2 with the final projection.
    V = (W2 @ Wfp.astype(np.float64)).astype(f)                 # [9,256,256]
    gt = type_embed @ Wft                                       # [14,256]
    gs = source_embed @ Wfs                                     # [5,256]
    gc = b2 @ Wfp + bf[None, :]                                 # [9,256]

    # W4 [128, T*512]: per expert t the four strip lhsT blocks
    #   [h0 | h1 | g0 | g1], each 128 cols.
    W4 = np.zeros((128, T * D * 2), dtype=f)
    VR = np.zeros((128, T * D * 2), dtype=f)
    for t in range(T):
        c = t * 512
        for h in range(2):
            W4[0:4, c + h * 128 : c + (h + 1) * 128] = W1[t][:, h * 128 : (h + 1) * 128]
            W4[4, c + h * 128 : c + (h + 1) * 128] = b1[t][h * 128 : (h + 1) * 128]
        for g in range(2):
            cg = c + 256 + g * 128
            W4[64 + 0, cg : cg + 128] = gc[t][g * 128 : (g + 1) * 128]
            W4[64 + 1 : 64 + 15, cg : cg + 128] = gt[:, g * 128 : (g + 1) * 128]
            W4[64 + 15 : 64 + 20, cg : cg + 128] = gs[:, g * 128 : (g + 1) * 128]
            # replicate for the 96-strip (g1 reads partitions 96..115)
            W4[96 + 0 : 96 + 20, cg : cg + 128] = W4[64 + 0 : 64 + 20, cg : cg + 128]
        # V chunks (h, g): rows = h dims, cols = out dims
        for h in range(2):
            for g in range(2):
                VR[:, c + h * 256 + g * 128 : c + h * 256 + (g + 1) * 128] = (
                    V[t][h * 128 : (h + 1) * 128, g * 128 : (g + 1) * 128])
    # strips 32/96 read their own partition rows; copy h1/g1 blocks there
    for t in range(T):
        c = t * 512
        W4[32:37, c + 128 : c + 256] = W4[0:5, c + 128 : c + 256]
        W4[0:5, c + 128 : c + 256] = 0.0
        W4[64 : 64 + 20, c + 384 : c + 512] = 0.0
    W4 = W4.reshape(128, T, 2, 256)
    W4a = np.ascontiguousarray(W4[0:37, :, 0, :])    # L1 strips (rows 0-36)
    W4b = np.ascontiguousarray(W4[64:120, :, 1, :])  # G strips (rows 64-119)
    return (W4a.astype(ml_dtypes.bfloat16), W4b.astype(ml_dtypes.bfloat16),
            VR.astype(ml_dtypes.bfloat16))


def _block_runs(m_t, L):
    """Per block: list of (c0, c1, expert) with cols relative to the block."""
    bounds = []
    off = 0
    for t in range(T):
        if m_t[t]:
            bounds.append((off, off + int(m_t[t]), t))
            off += int(m_t[t])
    if off < L:  # tail pad rides with the last expert
        bounds[-1] = (bounds[-1][0], L, bounds[-1][2])
    NB = L // BLOCK
    runs = [[] for _ in range(NB)]
    for (s0, s1, t) in bounds:
        b0, b1 = s0 // BLOCK, (s1 - 1) // BLOCK
        for b in range(b0, b1 + 1):
            c0 = max(s0 - b * BLOCK, 0)
            c1 = min(s1 - b * BLOCK, BLOCK)
            runs[b].append((c0, c1, t))
    return runs


def _build_program(m_t: tuple, L: int):
    """One compiled SPMD program for the given segment layout.

    Software-pipelined one block deep: emit slot(b+1) before V(b) so the
    PE never waits on the relu-h chain. PSUM: h pool 2x[128,512] (2
    banks), out pool 3x[128,1024] (6 banks) — the 3-deep out rotation
    keeps the G(b) -> V(b) -> DVE(b) -> G(b+3) buffer-reuse cycle off
    the critical path.
    """
    key = (m_t, L, _WARM_BURST, _FILL, _FILLW)
    if key in _PROGRAM_CACHE:
        return _PROGRAM_CACHE[key]

    NB = L // BLOCK
    NSB = (NB + 1) // 2
    runs = _block_runs(np.asarray(m_t, dtype=np.int64), L)

    nc = bacc.Bacc("TRN2", target_bir_lowering=False, debug=False,
                   num_devices=NCORES)
    xu_d = nc.dram_tensor("xu", [128, L], _BF16, kind="ExternalInput")
    w4a_d = nc.dram_tensor("w4a", [37, T, 256], _BF16, kind="ExternalInput")
    w4b_d = nc.dram_tensor("w4b", [56, T, 256], _BF16, kind="ExternalInput")
    vr_d = nc.dram_tensor("vr", [128, T * 512], _BF16, kind="ExternalInput")
    out_d = nc.dram_tensor("out", [128, NB * 1024], _BF16, kind="ExternalOutput")

    RELU = mybir.ActivationFunctionType.Relu

    with tile.TileContext(nc) as tc:
        with (
            tc.tile_pool(name="wts", bufs=1) as wts,
            tc.tile_pool(name="inp", bufs=6) as inp,
            tc.tile_pool(name="hsb", bufs=5) as hsbp,
            tc.tile_pool(name="osb", bufs=8) as osbp,
            tc.tile_pool(name="hps", bufs=2, space=bass.MemorySpace.PSUM) as hps,
            tc.tile_pool(name="ops", bufs=2, space=bass.MemorySpace.PSUM) as ops,
        ):
            w4 = wts.tile([128, T, 2, 256], _BF16)
            vr = wts.tile([128, T * 512], _BF16)

            # prime the ACT table (Relu) before the first real activation
            prime = wts.tile([1, 8], _BF16)
            nc.vector.memset(prime[:], 0.0)
            nc.scalar.activation(prime[0:1, 0:4], prime[0:1, 4:8], RELU)

            # bf16 warm-up burst: raise the PE HAM clock gate while the
            # first input/weight DMAs land
            if _WARM_BURST:
                wmw = wts.tile([128, 128], _BF16)
                wma = wts.tile([128, BLOCK], _BF16)
                nc.vector.memset(wmw[:], 0.0)
                nc.vector.memset(wma[:], 0.0)
                wmp = ops.tile([128, 1024], _F32, name="warmps", tag="o")
                for i in range(_WARM_BURST):
                    nc.tensor.matmul(wmp[:, 0:BLOCK], wmw[:], wma[:],
                                     start=True, stop=True)

            def emit_all_weights():
                """All expert chunks upfront on the sync queue (idle until
                the first out-DMA ~15us in), in first-use order."""
                seen = set()
                for b in range(NB):
                    for (c0, c1, t) in runs[b]:
                        if t not in seen:
                            seen.add(t)
                            nc.sync.dma_start(
                                w4[0:37, t : t + 1, 0:1, :],
                                w4a_d.ap()[:, t : t + 1, :])
                            nc.sync.dma_start(
                                w4[64:120, t : t + 1, 1:2, :],
                                w4b_d.ap()[:, t : t + 1, :])
                            nc.sync.dma_start(
                                vr[:, t * 512 : (t + 1) * 512],
                                vr_d.ap()[:, t * 512 : (t + 1) * 512])

            xuts = {}

            def emit_input_sb(sb):
                if sb >= NSB:
                    return
                # host pre-replicates all 4 strips: one DMA, no SBUF chain
                sbw = min(2 * BLOCK, L - sb * 1024)
                xut = inp.tile([128, 2 * BLOCK], _BF16, name=f"xu{sb}", tag="xu")
                nc.scalar.dma_start(xut[:, 0:sbw],
                                    xu_d.ap()[:, sb * 1024 : sb * 1024 + sbw])
                xuts[sb] = xut

            hpt = {}   # (b, h) -> [128,512] psum tile
            hst = {}   # b -> [128,1024] sbuf bf16 relu(h)
            opt = {}   # (b, g) -> [128,512] psum out accumulator
            ost = {}   # sb -> [128,2048] sbuf bf16 out staging

            def emit_l1(b):
                """L1 h0/h1 on row strips 0/32 + relu-h ACT ops."""
                if b >= NB:
                    return
                if b % 2 == 0:
                    emit_input_sb(b // 2 + 3)   # prefetch three superblocks out
                xut = xuts[b // 2]
                off = (b % 2) * BLOCK
                hpt[b] = hps.tile([128, 1024], _F32, name=f"h{b}", tag="h")
                for wi in range(2):
                    s = STRIPS[wi]
                    for (c0, c1, t) in runs[b]:
                        nc.tensor.matmul(
                            hpt[b][:, wi * BLOCK + c0 : wi * BLOCK + c1],
                            w4[s : s + K_L1, t, 0,
                               wi * 128 : wi * 128 + 128],
                            xut[s : s + K_L1, off + c0 : off + c1],
                            start=True, stop=True,
                            tile_position=(s, 0),
                        )
                # relu-h on DVE: it saturates (~1.22us/block) but its only
                # consumer V(b) is ~3 pairs downstream - huge slack
                hs = hsbp.tile([128, 1024], _BF16, name=f"hs{b}", tag="hs")
                nc.vector.tensor_scalar_max(hs[:], hpt[b][:], 0.0)
                hst[b] = hs

            def emit_v(b):
                """V accumulation; the first chunk per bank opens the group
                (the o-buf WAR reuse lands here, ~2 blocks of slack)."""
                opt[b] = ops.tile([128, 1024], _F32, name=f"o{b}", tag="o")
                # HAM keepers: tiny matmuls that close the PE idle window so
                # the clock gate stays at 8/8; overwritten by the real V
                # start=True group zeroing below.
                for fi in range(_FILL):
                    nc.tensor.matmul(opt[b][:, 0:_FILLW], vr[:, 0:128],
                                     hst[b][:, 0:_FILLW],
                                     start=True, stop=True,
                                     skip_group_check=True)
                started = set()
                for h in range(2):
                    for g in range(2):
                        for (c0, c1, t) in runs[b]:
                            vcol = t * 512 + h * 256 + g * 128
                            start = g not in started
                            started.add(g)
                            nc.tensor.matmul(
                                opt[b][:, g * BLOCK + c0 : g * BLOCK + c1],
                                vr[:, vcol : vcol + 128],
                                hst[b][:, h * BLOCK + c0 : h * BLOCK + c1],
                                start=start, stop=False,
                            )

            def emit_g_and_out(b):
                """G on strips 64/96 (concurrent with the next emit_l1's
                strips 0/32), closing each bank; then DVE out-relu + DMA."""
                xut = xuts[b // 2]
                off = (b % 2) * BLOCK
                nr = len(runs[b])
                for wi in (2, 3):
                    s = STRIPS[wi]
                    g = wi - 2
                    for i, (c0, c1, t) in enumerate(runs[b]):
                        nc.tensor.matmul(
                            opt[b][:, g * BLOCK + c0 : g * BLOCK + c1],
                            w4[s : s + K_G, t, 1,
                               g * 128 : g * 128 + 128],
                            xut[s : s + K_G, off + c0 : off + c1],
                            start=False, stop=(i == nr - 1),
                            tile_position=(s, 0),
                        )
                ost[b] = osbp.tile([128, 1024], _BF16, name=f"os{b}", tag="os")
                if b == NB - 1:
                    # tail: split relu + DMA so the first half ships while
                    # the second half is still in the scalar engine
                    for g in range(2):
                        nc.scalar.activation(
                            ost[b][:, g * BLOCK : (g + 1) * BLOCK],
                            opt[b][:, g * BLOCK : (g + 1) * BLOCK], RELU)
                        nc.sync.dma_start(
                            out_d.ap()[:, b * 1024 + g * BLOCK
                                       : b * 1024 + (g + 1) * BLOCK],
                            ost[b][:, g * BLOCK : (g + 1) * BLOCK])
                    return
                # out-relu on ACT (faster engine) - it sits in the o-buffer
                # reuse cycle, so its latency is on the critical path
                nc.scalar.activation(ost[b][:], opt[b][:], RELU)
                nc.sync.dma_start(out_d.ap()[:, b * 1024 : (b + 1) * 1024],
                                  ost[b][:])

            emit_all_weights()
            emit_input_sb(0)
            emit_input_sb(1)
            emit_input_sb(2)
            emit_l1(0)
            emit_l1(1)
            emit_l1(2)
            # mini-burst: keep the PE busy across the V(0) relu-h fill wait
            if _WARM_BURST:
                for i in range(3):
                    nc.tensor.matmul(wmp[:, 0:BLOCK], wmw[:], wma[:],
                                     start=True, stop=True)
            for b in range(NB):
                emit_v(b)
                emit_g_and_out(b)   # strips 64/96 ...
                emit_l1(b + 3)      # ... run concurrent with strips 0/32

    nc.compile()
    _PROGRAM_CACHE[key] = nc
    return nc


def kernel(type_ids, source_ids, params, type_embed, source_embed,
           W1, b1, W2, b2, Wf, bf):
    global LAST_RESULT
    type_ids = np.asarray(type_ids, dtype=np.int32)
    source_ids = np.asarray(source_ids, dtype=np.int32)
    params = np.asarray(params, dtype=np.float32)
    E = type_ids.shape[0]

    base_ids = BASE_MAP[type_ids]
    n_t, m_t, L = _layout(base_ids)
    ORD = _build_order(base_ids, n_t, m_t, L)
    XU = _host_inputs(type_ids, source_ids, params, ORD)
    W4a, W4b, VR = _host_weights(
        np.asarray(type_embed), np.asarray(source_embed),
        np.asarray(W1), np.asarray(b1), np.asarray(W2), np.asarray(b2),
        np.asarray(Wf), np.asarray(bf))

    nc = _build_program(tuple(int(v) for v in m_t), L)

    in_maps = []
    for c in range(NCORES):
        in_maps.append({"xu": np.ascontiguousarray(XU[c]),
                        "w4a": W4a, "w4b": W4b, "vr": VR})

    trace = bool(int(os.environ.get("EDGEENC_TRACE", "0")))
    res = run_bass_kernel_spmd(nc, in_maps, core_ids=list(range(NCORES)),
                               trace=trace)
    LAST_RESULT = res

    NB = L // BLOCK
    full = np.zeros((E, D), dtype=np.float32)
    for c in range(NCORES):
        oc = res.results[c]["out"]                     # [128, NB*1024] bf16
        oc = np.asarray(oc)
        if oc.dtype != np.float32:
            oc = oc.astype(np.float32)
        # cols: [block b][g half][512 edges] -> [D, L]
        oc = oc.reshape(128, NB, 2, BLOCK)             # p, b, g, e
        oc = oc.transpose(2, 0, 1, 3).reshape(D, L)    # d = g*128+p
        sel = ORD[c] >= 0
        full[ORD[c][sel]] = np.ascontiguousarray(oc[:, sel].T)
    return full



# revision 18
# speedup vs baseline: 1.1739x; 1.1739x over previous
"""Trainium2 Bass kernel for nn_EdgeEncoder (moe_routing).

Strategy
--------
Each of E edges is routed to 1 of 9 expert MLPs (4 -> 256 -> 256), then
  out = relu(concat([type_embed[tid], source_embed[sid], pv]) @ Wf + bf).

Host (numpy, cheap O(E) work):
  * scale/mask params, group edge indices by expert (base type) at
    256-edge granularity, split evenly over 8 cores (identical layout on
    every core so one SPMD program serves all 8),
  * algebraic fusions: b1 rides a ones-row inside layer 1;
    V[t] = W2[t] @ Wf_pv fuses layer 2 with the final projection;
    G[t] = [const; type_embed@Wf_t; source_embed@Wf_s] turns the
    embedding gathers + all biases into one small matmul against the
    one-hot rows.

Device, all bf16 operands (fp32 PSUM accumulate; rel-err gate is 2e-2):
  The per-512-edge block needs h = relu(W1e.T @ x1) (2 matmuls, K=5),
  the G part (2 matmuls, K=20) and V part (4 matmuls, K=128) of the
  output. The four small-K matmuls are packed into ONE matmul slot via
  tile_position row-strips (0/32/64/96): the host ships a [32, L] input
  with per-strip band contents (x+ones for the two L1 strips,
  ones+one-hots for the two K=20 G strips), and the strip matmuls run
  concurrently in the PE array. Per block: 1 packed slot + 4 V matmuls
  + a tiny HAM-keeper matmul that holds the PE clock gate at 8/8.
  Weights ship compactly (W4a rows 0-36, W4b rows 64-119) and deep SBUF
  pools (inp 6 / hsb 5 / osb 8) decouple the pipeline from DMA latency.
  relu-h is one [128,1024] ACT op, out-relu one [128,1024] DVE op
  (PSUM->SBUF, the only engines that can read PSUM). Outputs are stored
  bf16 in a DMA-native packed layout and unscrambled on host.
"""

import math
import os

import ml_dtypes
import numpy as np

import concourse.bacc as bacc
import concourse.bass as bass
import concourse.mybir as mybir
import concourse.tile as tile
from concourse.bass_utils import run_bass_kernel_spmd

# ---- static module configuration (mirrors the torch source) ----
T = 9            # base types ("experts")
P_MAX = 4
D = 256
N_TYPES = 14
N_SRC = 5
NCORES = 8
BLOCK = 512      # edges per block (one PSUM bank per 128-out-dims half)
GRAN = 1         # run granularity (expert segments padded to per-core exact)

BASE_MAP = np.array([0, 0, 0, 1, 1, 1, 2, 2, 3, 4, 5, 6, 7, 8], dtype=np.int32)
PCOUNT = np.array([2, 2, 1, 1, 1, 1, 3, 2, 4], dtype=np.int32)
SCALES = np.ones((T, P_MAX), dtype=np.float32)
SCALES[0, :2] = [1.0, 1e-06]      # nmos  m, w
SCALES[1, :2] = [1.0, 1e-06]      # pmos  m, w
SCALES[2, 0] = 1.0                # balun rout
SCALES[3, 0] = 1000.0             # resistor r
SCALES[4, 0] = 1e-12              # capacitor c
SCALES[5, 0] = 1e-09              # inductor l
SCALES[6, :3] = [1.0, 1.0, 1.0]   # vsource dc, mag, phase
SCALES[7, :2] = [0.001, 0.001]    # isource dc, mag
SCALES[8, :4] = [1.0, 1.0, 1e9, 1.0]  # port dbm, dc, freq, num

# xu strip layout (replicated at partition offsets 0/32/64/96):
#   rows 0-3: scaled params, row 4: ones (valid), rows 5-18: type one-hot,
#   rows 19-23: source one-hot, rows 24-31: zero
K_L1 = 5                   # x rows + ones
K_G = 20                   # G bands: ones + type one-hot + src one-hot
STRIPS = (0, 32, 64, 96)   # (L1 h0, L1 h1, G g0, G g1)

_F32 = mybir.dt.float32
_BF16 = mybir.dt.bfloat16
_WARM_BURST = int(os.environ.get("EDGEENC_WARM_BURST", "6"))
_FILL = int(os.environ.get("EDGEENC_FILL", "1"))
_FILLW = int(os.environ.get("EDGEENC_FILLW", "96"))

_PROGRAM_CACHE: dict = {}
LAST_RESULT = None  # BassKernelResults of the most recent run (for test harness)


def _layout(base_ids: np.ndarray):
    """Per-expert per-core segment sizes (multiples of GRAN), identical on
    every core so one program serves all 8."""
    n_t = np.bincount(base_ids, minlength=T)
    m_t = np.zeros(T, dtype=np.int64)
    for t in range(T):
        if n_t[t] > 0:
            per_core = math.ceil(n_t[t] / NCORES)
            m_t[t] = math.ceil(per_core / GRAN) * GRAN
    L0 = int(m_t.sum())
    L = math.ceil(L0 / BLOCK) * BLOCK
    # fold the tail pad into the last present expert's segment
    last = int(np.nonzero(m_t)[0][-1])
    m_t[last] += L - L0
    return n_t, m_t, L


def _build_order(base_ids: np.ndarray, n_t, m_t, L) -> np.ndarray:
    """ORD[c, j] = global edge index at per-core slot j (or -1 = pad)."""
    ORD = np.full((NCORES, L), -1, dtype=np.int64)
    off = 0
    for t in range(T):
        if m_t[t] == 0:
            continue
        seg = int(m_t[t])
        idx = np.nonzero(base_ids == t)[0]
        arr = np.full(NCORES * seg, -1, dtype=np.int64)
        arr[: idx.shape[0]] = idx
        ORD[:, off : off + seg] = arr.reshape(NCORES, seg)
        off += seg
    return ORD


def _host_inputs(type_ids, source_ids, params, ORD):
    """XU[c] = [32, L] bf16: x rows, ones, type one-hot, src one-hot, zeros."""
    base_ids = BASE_MAP[type_ids]
    scales = SCALES[base_ids]                                  # [E,4]
    validp = np.arange(P_MAX)[None, :] < PCOUNT[base_ids][:, None]
    x = np.where(validp, params.astype(np.float32) / scales, 0.0).astype(np.float32)

    L = ORD.shape[1]
    XU = np.zeros((NCORES, 128, L), dtype=np.float32)
    valid = ORD >= 0
    ids = ORD[valid]
    tmp = np.zeros((NCORES, L, P_MAX), dtype=np.float32)
    tmp[valid] = x[ids]
    # bands 0/1 (L1 strips): x rows + ones
    XU[:, 0:P_MAX, :] = tmp.transpose(0, 2, 1)
    XU[:, P_MAX, :] = valid
    XU[:, 32:37, :] = XU[:, 0:5, :]
    # bands 2/3 (G strips): ones + type one-hot + src one-hot
    XU[:, 64, :] = valid
    ci, co = np.nonzero(valid)
    XU[ci, 65 + type_ids[ids], co] = 1.0
    XU[ci, 79 + source_ids[ids], co] = 1.0
    XU[:, 96:116, :] = XU[:, 64:84, :]
    return XU.astype(ml_dtypes.bfloat16)


def _host_weights(type_embed, source_embed, W1, b1, W2, b2, Wf, bf):
    f = np.float32
    W1 = W1.astype(f); b1 = b1.astype(f); W2 = W2.astype(np.float64)
    b2 = b2.astype(f); Wf = Wf.astype(f); bf = bf.astype(f)
    type_embed = type_embed.astype(f); source_embed = source_embed.astype(f)

    Wft, Wfs, Wfp = Wf[:D], Wf[D : 2 * D], Wf[2 * D :]
    # V[t] = W2[t] @ Wf_pv (f64), fusing layer 2 with the final projection.
    V = (W2 @ Wfp.astype(np.float64)).astype(f)                 # [9,256,256]
    gt = type_embed @ Wft                                       # [14,256]
    gs = source_embed @ Wfs                                     # [5,256]
    gc = b2 @ Wfp + bf[None, :]                                 # [9,256]

    # W4 [128, T*512]: per expert t the four strip lhsT blocks
    #   [h0 | h1 | g0 | g1], each 128 cols.
    W4 = np.zeros((128, T * D * 2), dtype=f)
    VR = np.zeros((128, T * D * 2), dtype=f)
    for t in range(T):
        c = t * 512
        for h in range(2):
            W4[0:4, c + h * 128 : c + (h + 1) * 128] = W1[t][:, h * 128 : (h + 1) * 128]
            W4[4, c + h * 128 : c + (h + 1) * 128] = b1[t][h * 128 : (h + 1) * 128]
        for g in range(2):
            cg = c + 256 + g * 128
            W4[64 + 0, cg : cg + 128] = gc[t][g * 128 : (g + 1) * 128]
            W4[64 + 1 : 64 + 15, cg : cg + 128] = gt[:, g * 128 : (g + 1) * 128]
            W4[64 + 15 : 64 + 20, cg : cg + 128] = gs[:, g * 128 : (g + 1) * 128]
            # replicate for the 96-strip (g1 reads partitions 96..115)
            W4[96 + 0 : 96 + 20, cg : cg + 128] = W4[64 + 0 : 64 + 20, cg : cg + 128]
        # V chunks (h, g): rows = h dims, cols = out dims
        for h in range(2):
            for g in range(2):
                VR[:, c + h * 256 + g * 128 : c + h * 256 + (g + 1) * 128] = (
                    V[t][h * 128 : (h + 1) * 128, g * 128 : (g + 1) * 128])
    # strips 32/96 read their own partition rows; copy h1/g1 blocks there
    for t in range(T):
        c = t * 512
        W4[32:37, c + 128 : c + 256] = W4[0:5, c + 128 : c + 256]
        W4[0:5, c + 128 : c + 256] = 0.0
        W4[64 : 64 + 20, c + 384 : c + 512] = 0.0
    W4 = W4.reshape(128, T, 2, 256)
    W4a = np.ascontiguousarray(W4[0:37, :, 0, :])    # L1 strips (rows 0-36)
    W4b = np.ascontiguousarray(W4[64:120, :, 1, :])  # G strips (rows 64-119)
    return (W4a.astype(ml_dtypes.bfloat16), W4b.astype(ml_dtypes.bfloat16),
            VR.astype(ml_dtypes.bfloat16))


def _block_runs(m_t, L):
    """Per block: list of (c0, c1, expert) with cols relative to the block."""
    bounds = []
    off = 0
    for t in range(T):
        if m_t[t]:
            bounds.append((off, off + int(m_t[t]), t))
            off += int(m_t[t])
    if off < L:  # tail pad rides with the last expert
        bounds[-1] = (bounds[-1][0], L, bounds[-1][2])
    NB = L // BLOCK
    runs = [[] for _ in range(NB)]
    for (s0, s1, t) in bounds:
        b0, b1 = s0 // BLOCK, (s1 - 1) // BLOCK
        for b in range(b0, b1 + 1):
            c0 = max(s0 - b * BLOCK, 0)
            c1 = min(s1 - b * BLOCK, BLOCK)
            runs[b].append((c0, c1, t))
    return runs


def _build_program(m_t: tuple, L: int):
    """One compiled SPMD program for the given segment layout.

    Software-pipelined one block deep: emit slot(b+1) before V(b) so the
    PE never waits on the relu-h chain. PSUM: h pool 2x[128,512] (2
    banks), out pool 3x[128,1024] (6 banks) — the 3-deep out rotation
    keeps the G(b) -> V(b) -> DVE(b) -> G(b+3) buffer-reuse cycle off
    the critical path.
    """
    key = (m_t, L, _WARM_BURST, _FILL, _FILLW)
    if key in _PROGRAM_CACHE:
        return _PROGRAM_CACHE[key]

    NB = L // BLOCK
    NSB = (NB + 1) // 2
    runs = _block_runs(np.asarray(m_t, dtype=np.int64), L)

    nc = bacc.Bacc("TRN2", target_bir_lowering=False, debug=False,
                   num_devices=NCORES)
    xu_d = nc.dram_tensor("xu", [128, L], _BF16, kind="ExternalInput")
    w4a_d = nc.dram_tensor("w4a", [37, T, 256], _BF16, kind="ExternalInput")
    w4b_d = nc.dram_tensor("w4b", [56, T, 256], _BF16, kind="ExternalInput")
    vr_d = nc.dram_tensor("vr", [128, T * 512], _BF16, kind="ExternalInput")
    out_d = nc.dram_tensor("out", [128, NB * 1024], _BF16, kind="ExternalOutput")

    RELU = mybir.ActivationFunctionType.Relu

    with tile.TileContext(nc) as tc:
        with (
            tc.tile_pool(name="wts", bufs=1) as wts,
            tc.tile_pool(name="inp", bufs=6) as inp,
            tc.tile_pool(name="hsb", bufs=5) as hsbp,
            tc.tile_pool(name="osb", bufs=8) as osbp,
            tc.tile_pool(name="hps", bufs=2, space=bass.MemorySpace.PSUM) as hps,
            tc.tile_pool(name="ops", bufs=2, space=bass.MemorySpace.PSUM) as ops,
        ):
            w4 = wts.tile([128, T, 2, 256], _BF16)
            vr = wts.tile([128, T * 512], _BF16)

            # prime the ACT table (Relu) before the first real activation
            prime = wts.tile([1, 8], _BF16)
            nc.vector.memset(prime[:], 0.0)
            nc.scalar.activation(prime[0:1, 0:4], prime[0:1, 4:8], RELU)

            # bf16 warm-up burst: raise the PE HAM clock gate while the
            # first input/weight DMAs land
            if _WARM_BURST:
                wmw = wts.tile([128, 128], _BF16)
                wma = wts.tile([128, BLOCK], _BF16)
                nc.vector.memset(wmw[:], 0.0)
                nc.vector.memset(wma[:], 0.0)
                wmp = ops.tile([128, 1024], _F32, name="warmps", tag="o")
                for i in range(_WARM_BURST):
                    nc.tensor.matmul(wmp[:, 0:BLOCK], wmw[:], wma[:],
                                     start=True, stop=True)

            def emit_all_weights():
                """All expert chunks upfront on the sync queue (idle until
                the first out-DMA ~15us in), in first-use order."""
                seen = set()
                for b in range(NB):
                    for (c0, c1, t) in runs[b]:
                        if t not in seen:
                            seen.add(t)
                            nc.sync.dma_start(
                                w4[0:37, t : t + 1, 0:1, :],
                                w4a_d.ap()[:, t : t + 1, :])
                            nc.sync.dma_start(
                                w4[64:120, t : t + 1, 1:2, :],
                                w4b_d.ap()[:, t : t + 1, :])
                            nc.sync.dma_start(
                                vr[:, t * 512 : (t + 1) * 512],
                                vr_d.ap()[:, t * 512 : (t + 1) * 512])

            xuts = {}

            def emit_input_sb(sb):
                if sb >= NSB:
                    return
                # host pre-replicates all 4 strips: one DMA, no SBUF chain
                sbw = min(2 * BLOCK, L - sb * 1024)
                xut = inp.tile([128, 2 * BLOCK], _BF16, name=f"xu{sb}", tag="xu")
                nc.gpsimd.dma_start(xut[:, 0:sbw],
                                    xu_d.ap()[:, sb * 1024 : sb * 1024 + sbw])
                xuts[sb] = xut

            hpt = {}   # (b, h) -> [128,512] psum tile
            hst = {}   # b -> [128,1024] sbuf bf16 relu(h)
            opt = {}   # (b, g) -> [128,512] psum out accumulator
            ost = {}   # sb -> [128,2048] sbuf bf16 out staging

            def emit_l1(b):
                """L1 h0/h1 on row strips 0/32 + relu-h ACT ops."""
                if b >= NB:
                    return
                if b % 2 == 0:
                    emit_input_sb(b // 2 + 3)   # prefetch three superblocks out
                xut = xuts[b // 2]
                off = (b % 2) * BLOCK
                hpt[b] = hps.tile([128, 1024], _F32, name=f"h{b}", tag="h")
                for wi in range(2):
                    s = STRIPS[wi]
                    for (c0, c1, t) in runs[b]:
                        nc.tensor.matmul(
                            hpt[b][:, wi * BLOCK + c0 : wi * BLOCK + c1],
                            w4[s : s + K_L1, t, 0,
                               wi * 128 : wi * 128 + 128],
                            xut[s : s + K_L1, off + c0 : off + c1],
                            start=True, stop=True,
                            tile_position=(s, 0),
                        )
                # relu-h on DVE: it saturates (~1.22us/block) but its only
                # consumer V(b) is ~3 pairs downstream - huge slack
                hs = hsbp.tile([128, 1024], _BF16, name=f"hs{b}", tag="hs")
                if b == NB - 1:
                    # tail: split so the V h0 matmuls start half an op earlier
                    for wi in range(2):
                        nc.vector.tensor_scalar_max(
                            hs[:, wi * BLOCK : (wi + 1) * BLOCK],
                            hpt[b][:, wi * BLOCK : (wi + 1) * BLOCK], 0.0)
                else:
                    nc.vector.tensor_scalar_max(hs[:], hpt[b][:], 0.0)
                hst[b] = hs

            def emit_v(b):
                """V accumulation; the first chunk per bank opens the group
                (the o-buf WAR reuse lands here, ~2 blocks of slack)."""
                opt[b] = ops.tile([128, 1024], _F32, name=f"o{b}", tag="o")
                # HAM keepers: tiny matmuls that close the PE idle window so
                # the clock gate stays at 8/8; overwritten by the real V
                # start=True group zeroing below.
                for fi in range(_FILL):
                    nc.tensor.matmul(opt[b][:, 0:_FILLW], vr[:, 0:128],
                                     hst[b][:, 0:_FILLW],
                                     start=True, stop=True,
                                     skip_group_check=True)
                started = set()
                for h in range(2):
                    for g in range(2):
                        for (c0, c1, t) in runs[b]:
                            vcol = t * 512 + h * 256 + g * 128
                            start = g not in started
                            started.add(g)
                            nc.tensor.matmul(
                                opt[b][:, g * BLOCK + c0 : g * BLOCK + c1],
                                vr[:, vcol : vcol + 128],
                                hst[b][:, h * BLOCK + c0 : h * BLOCK + c1],
                                start=start, stop=False,
                            )

            def emit_g_and_out(b):
                """G on strips 64/96 (concurrent with the next emit_l1's
                strips 0/32), closing each bank; then DVE out-relu + DMA."""
                xut = xuts[b // 2]
                off = (b % 2) * BLOCK
                nr = len(runs[b])
                for wi in (2, 3):
                    s = STRIPS[wi]
                    g = wi - 2
                    for i, (c0, c1, t) in enumerate(runs[b]):
                        nc.tensor.matmul(
                            opt[b][:, g * BLOCK + c0 : g * BLOCK + c1],
                            w4[s : s + K_G, t, 1,
                               g * 128 : g * 128 + 128],
                            xut[s : s + K_G, off + c0 : off + c1],
                            start=False, stop=(i == nr - 1),
                            tile_position=(s, 0),
                        )
                ost[b] = osbp.tile([128, 1024], _BF16, name=f"os{b}", tag="os")
                if b == NB - 1:
                    # tail: quarter-granular relu + DMA so chunks ship while
                    # later quarters are still in the scalar engine
                    Q = BLOCK // 2
                    for q in range(4):
                        nc.scalar.activation(
                            ost[b][:, q * Q : (q + 1) * Q],
                            opt[b][:, q * Q : (q + 1) * Q], RELU)
                        nc.sync.dma_start(
                            out_d.ap()[:, b * 1024 + q * Q
                                       : b * 1024 + (q + 1) * Q],
                            ost[b][:, q * Q : (q + 1) * Q])
                    return
                # out-relu on ACT (faster engine) - it sits in the o-buffer
                # reuse cycle, so its latency is on the critical path
                nc.scalar.activation(ost[b][:], opt[b][:], RELU)
                nc.sync.dma_start(out_d.ap()[:, b * 1024 : (b + 1) * 1024],
                                  ost[b][:])

            emit_all_weights()
            emit_input_sb(0)
            emit_input_sb(1)
            emit_input_sb(2)
            emit_l1(0)
            emit_l1(1)
            emit_l1(2)
            # mini-burst: keep the PE busy across the V(0) relu-h fill wait
            if _WARM_BURST:
                for i in range(3):
                    nc.tensor.matmul(wmp[:, 0:BLOCK], wmw[:], wma[:],
                                     start=True, stop=True)
            for b in range(NB):
                emit_v(b)
                emit_g_and_out(b)   # strips 64/96 ...
                emit_l1(b + 3)      # ... run concurrent with strips 0/32

    nc.compile()
    _PROGRAM_CACHE[key] = nc
    return nc


def kernel(type_ids, source_ids, params, type_embed, source_embed,
           W1, b1, W2, b2, Wf, bf):
    global LAST_RESULT
    type_ids = np.asarray(type_ids, dtype=np.int32)
    source_ids = np.asarray(source_ids, dtype=np.int32)
    params = np.asarray(params, dtype=np.float32)
    E = type_ids.shape[0]

    base_ids = BASE_MAP[type_ids]
    n_t, m_t, L = _layout(base_ids)
    ORD = _build_order(base_ids, n_t, m_t, L)
    XU = _host_inputs(type_ids, source_ids, params, ORD)
    W4a, W4b, VR = _host_weights(
        np.asarray(type_embed), np.asarray(source_embed),
        np.asarray(W1), np.asarray(b1), np.asarray(W2), np.asarray(b2),
        np.asarray(Wf), np.asarray(bf))

    nc = _build_program(tuple(int(v) for v in m_t), L)

    in_maps = []
    for c in range(NCORES):
        in_maps.append({"xu": np.ascontiguousarray(XU[c]),
                        "w4a": W4a, "w4b": W4b, "vr": VR})

    trace = bool(int(os.environ.get("EDGEENC_TRACE", "0")))
    res = run_bass_kernel_spmd(nc, in_maps, core_ids=list(range(NCORES)),
                               trace=trace)
    LAST_RESULT = res

    NB = L // BLOCK
    full = np.zeros((E, D), dtype=np.float32)
    for c in range(NCORES):
        oc = res.results[c]["out"]                     # [128, NB*1024] bf16
        oc = np.asarray(oc)
        if oc.dtype != np.float32:
            oc = oc.astype(np.float32)
        # cols: [block b][g half][512 edges] -> [D, L]
        oc = oc.reshape(128, NB, 2, BLOCK)             # p, b, g, e
        oc = oc.transpose(2, 0, 1, 3).reshape(D, L)    # d = g*128+p
        sel = ORD[c] >= 0
        full[ORD[c][sel]] = np.ascontiguousarray(oc[:, sel].T)
    return full

